# revision 1
# baseline (speedup 1.0000x reference)
"""Trainium2 Bass kernel for GQA attention (nn_Attention_40364102648437).

Problem: B=2, S=2048, HIDDEN=896, 14 q heads / 2 kv heads, head_dim 64,
RoPE (theta 1e6), causal softmax, o-projection.

Sharding (8 cores, SPMD): core = b*4 + kv*2 + half.
Each core owns one batch b, one kv head, and 4 q-head slots (7 q heads per
kv group are split 4+3; the last slot of the second half is a duplicate
whose wo rows are zeroed so its contribution vanishes). Every core computes
a full [S, HIDDEN] partial output (its heads' contribution through wo);
the host sums the 4 partials per batch.

Matmuls run in float32r (fp32 rounded to 11 mantissa bits, full PE rate at
free dim >= 256). The walrus verifier requires every fp32r matmul input to
be produced by an instruction that rounds to fp32r, so matmul-input DRAM
tensors are declared float32r (host pre-rounds the bits) and on-chip
producers (rope final add, exp activation, v copy, normalization multiply)
write float32r-typed tiles directly.

On-core layout: everything is kept "transposed" ([feature, seq]):
  qT/kT tiles [128, S] hold two head-slots stacked (rows 0-63 / 64-127),
  scores are computed as S^T [k_pos, q_pos] via row-paired K=64 matmuls
  (partition bases 0/64 -> PE row groups run concurrently),
  exp() runs on ScalarE with no max subtraction (scores are O(1)),
  V is transposed on the PE once per k-block and augmented with a ones
  column so each PV matmul also produces the softmax denominator, and the
  o-projection consumes the [feature, seq] attention output directly as
  the stationary operand.

Hardware constraints found the hard way (device crashes / wrong results):
  - concurrent row-group matmuls (tile_position rows 0/64) must write
    DIFFERENT PSUM banks -> s_ps layout puts slot a in bank 0, slot b in
    bank 1;
  - matmul start=True clears has_written for the WHOLE PSUM bank, so a
    tile holding several accumulation regions gets exactly one start/stop;
  - engines cannot move data across partitions: RoPE's rotate-half swap
    and the slot-b restack use SBUF->SBUF DMA, and the softmax 1/Z row is
    replicated across partitions with a DRAM-bounce broadcast DMA
    (SBUF-source broadcast APs are rejected);
  - tensor_tensor may read only one input from PSUM.
"""

import os

import numpy as np

import concourse.bass as bass
import concourse.mybir as mybir
from concourse import bacc
from concourse.tile import TileContext
from concourse.masks import make_identity
from concourse.bass_utils import run_bass_kernel_spmd

F32 = mybir.dt.float32
F32R = mybir.dt.float32r
BF16 = mybir.dt.bfloat16

HIDDEN = 896
HEAD_DIM = 64
B = 2
S = 2048
ROPE_THETA = 1000000.0
NH7 = HIDDEN // 128  # 7 hidden tiles
NKB = S // 128       # 16 key blocks
NJ = S // 256        # 8 query superblocks (256 q positions each)
MASK_VAL = -1e9


def build_program():
    phases = os.environ.get("K_PHASES", "ABC")
    nc = bacc.Bacc("TRN2", target_bir_lowering=False, debug=False, num_devices=8)

    # host-pre-tiled: row ss*128+p holds [t, n] -> hs[b][ss*512+n, t*128+p]
    hsT = nc.dram_tensor("hsT", [4 * 128, NH7 * 512], F32R, kind="ExternalInput")
    wq4 = nc.dram_tensor("wq4", [HIDDEN, 256], F32R, kind="ExternalInput")
    bq4 = nc.dram_tensor("bq4", [2, 128], F32, kind="ExternalInput")
    wkv = nc.dram_tensor("wkv", [HIDDEN, 128], F32R, kind="ExternalInput")
    bkv = nc.dram_tensor("bkv", [1, 128], F32, kind="ExternalInput")
    wo4 = nc.dram_tensor("wo4", [256, HIDDEN], F32R, kind="ExternalInput")
    cosd = nc.dram_tensor("cosd", [128, S], F32, kind="ExternalInput")
    sind = nc.dram_tensor("sind", [128, S], F32, kind="ExternalInput")
    maskD = nc.dram_tensor("maskD", [128, 1024], F32, kind="ExternalInput")
    out_d = nc.dram_tensor("out", [S, HIDDEN], F32, kind="ExternalOutput")

    EXP = mybir.ActivationFunctionType.Exp

    with TileContext(nc) as tc:
        with (
            tc.tile_pool(name="const", bufs=1) as cpool,
            tc.tile_pool(name="big", bufs=1) as bigpool,
        ):
            # ---- constants
            wkv_sb = cpool.tile([128, NH7 * 128], F32R)
            for h in range(NH7):
                nc.sync.dma_start(
                    out=wkv_sb[:, h * 128 : (h + 1) * 128],
                    in_=wkv[h * 128 : (h + 1) * 128, :],
                )
            wq_sb = cpool.tile([128, NH7 * 256], F32R)
            for h in range(NH7):
                nc.sync.dma_start(
                    out=wq_sb[:, h * 256 : (h + 1) * 256],
                    in_=wq4[h * 128 : (h + 1) * 128, :],
                )
            wo_sb = cpool.tile([128, 2 * HIDDEN], F32R)
            for ft in range(2):
                nc.sync.dma_start(
                    out=wo_sb[:, ft * HIDDEN : (ft + 1) * HIDDEN],
                    in_=wo4[ft * 128 : (ft + 1) * 128, :],
                )
            cos_sb = cpool.tile([128, S], F32)
            nc.sync.dma_start(out=cos_sb[:], in_=cosd[:])
            sin_sb = cpool.tile([128, S], F32)
            nc.sync.dma_start(out=sin_sb[:], in_=sind[:])
            mask_sb = cpool.tile([128, 1024], F32)
            nc.sync.dma_start(out=mask_sb[:], in_=maskD[:])
            bq_sb = cpool.tile([128, 2], F32)
            nc.sync.dma_start(out=bq_sb[:], in_=bq4.rearrange("a p -> p a"))
            bkv_sb = cpool.tile([128, 1], F32)
            nc.sync.dma_start(out=bkv_sb[:], in_=bkv.rearrange("a p -> p a"))
            ident = cpool.tile([128, 128], F32)
            make_identity(nc, ident[:])
            ones_f32 = cpool.tile([128, 64], F32)
            nc.vector.memset(ones_f32[:], 1.0)

            # ---- persistent activations
            qA = bigpool.tile([128, S], F32)
            qB = bigpool.tile([128, S], F32)
            kvT = bigpool.tile([128, S], F32)
            kdup = bigpool.tile([128, S], F32)
            qAr = bigpool.tile([128, S], F32R)
            qBr = bigpool.tile([128, S], F32R)
            kdr = bigpool.tile([128, S], F32R)
            v_sb = bigpool.tile([128, NKB * 65], F32R)
            aoT0 = bigpool.tile([128, S], F32R)
            aoT1 = bigpool.tile([128, S], F32R)
            stg0 = bigpool.tile([64, S], F32R)
            stg1 = bigpool.tile([64, S], F32R)

            # ================= phase A: projections =================
            # kv projections first (k/v gate the attention start); hs tiles
            # stay resident so q projections reuse them without reloading
            with (
                tc.tile_pool(name="hst", bufs=4) as hpool,
                tc.tile_pool(name="pps", bufs=2, space="PSUM") as ppool,
            ):
                hs_tiles = []
                for ss in range(4):
                    ssl = slice(ss * 512, (ss + 1) * 512)
                    hs_t = hpool.tile([128, NH7 * 512], F32R)
                    hs_tiles.append(hs_t)
                    eng = (nc.sync, nc.scalar)[ss % 2]
                    eng.dma_start(
                        out=hs_t[:], in_=hsT[ss * 128 : (ss + 1) * 128, :]
                    )
                    kv_ps = ppool.tile([128, 512], F32)
                    for h in range(NH7):
                        nc.tensor.matmul(
                            kv_ps[:],
                            wkv_sb[:, h * 128 : (h + 1) * 128],
                            hs_t[:, h * 512 : (h + 1) * 512],
                            start=(h == 0),
                            stop=(h == NH7 - 1),
                        )
                    nc.vector.tensor_scalar_add(kvT[:, ssl], kv_ps[:], bkv_sb[:, 0:1])
                for ss in range(4):
                    ssl = slice(ss * 512, (ss + 1) * 512)
                    hs_t = hs_tiles[ss]
                    for ft in range(2):
                        q_ps = ppool.tile([128, 512], F32)
                        for h in range(NH7):
                            nc.tensor.matmul(
                                q_ps[:],
                                wq_sb[:, h * 256 + ft * 128 : h * 256 + (ft + 1) * 128],
                                hs_t[:, h * 512 : (h + 1) * 512],
                                start=(h == 0),
                                stop=(h == NH7 - 1),
                            )
                        qt = (qA, qB)[ft]
                        nc.vector.tensor_scalar_add(
                            qt[:, ssl], q_ps[:], bq_sb[:, ft : ft + 1]
                        )

            # ---- duplicate kT into both partition halves
            nc.vector.tensor_copy(kdup[0:64, :], kvT[0:64, :])
            nc.sync.dma_start(out=kdup[64:128, :], in_=kvT[0:64, :])

            # ---- RoPE on qA, qB, kdup -> rounded fp32r tiles, chunked
            # column-wise so attention can start after the first chunk
            with tc.tile_pool(name="swp", bufs=2) as swpool:
                for t, tr in ((kdup, kdr), (qA, qAr), (qB, qBr)):
                    tsw = swpool.tile([128, S], F32)
                    for c in range(2):
                        csl = slice(c * 1024, (c + 1) * 1024)
                        for dst, src in ((0, 32), (32, 0), (64, 96), (96, 64)):
                            nc.sync.dma_start(
                                out=tsw[dst : dst + 32, csl],
                                in_=t[src : src + 32, csl],
                            )
                        nc.vector.tensor_mul(tsw[:, csl], tsw[:, csl], sin_sb[:, csl])
                        nc.vector.tensor_mul(t[:, csl], t[:, csl], cos_sb[:, csl])
                        nc.vector.tensor_add(tr[:, csl], t[:, csl], tsw[:, csl])

            # ---- v natural layout [k_pos, 64] + ones column (col 64 of 65)
            with tc.tile_pool(name="vtr", bufs=2, space="PSUM") as vpool:
                for kb in range(NKB):
                    vt_ps = vpool.tile([128, 64], F32)
                    nc.tensor.transpose(
                        vt_ps[:],
                        kvT[64:128, kb * 128 : (kb + 1) * 128],
                        ident[64:128, 64:128],
                    )
                    nc.vector.tensor_copy(v_sb[:, kb * 65 : kb * 65 + 64], vt_ps[:])
                    nc.vector.tensor_copy(
                        v_sb[:, kb * 65 + 64 : kb * 65 + 65], ones_f32[:, 0:1]
                    )

            if "B" not in phases:
                nc.sync.dma_start(out=out_d[0:128, :], in_=kdr[:, 0:HIDDEN].bitcast(F32))
                nc.sync.dma_start(out=out_d[128:256, :], in_=qAr[:, 0:HIDDEN].bitcast(F32))
                nc.sync.dma_start(out=out_d[256:384, :], in_=v_sb[:, 0:HIDDEN].bitcast(F32))

            # ================= phase B: attention =================
            with (
                tc.tile_pool(name="sps", bufs=2, space="PSUM") as spool,
                tc.tile_pool(name="ops", bufs=2, space="PSUM") as opool,
                tc.tile_pool(name="fps", bufs=1, space="PSUM") as fpool,
                tc.tile_pool(name="esb", bufs=4) as epool,
                tc.tile_pool(name="rcs", bufs=3) as rcpool,
                tc.tile_pool(name="osb", bufs=4) as obpool,
                tc.tile_pool(name="zbp", bufs=3, space="DRAM") as zbpool,
            ):
                blvl = int(os.environ.get("K_BLVL", "3"))
                for J in range(NJ if "B" in phases else 0):
                    for pair in range(2):
                        qt = (qAr, qBr)[pair]
                        aoT = (aoT0, aoT1)[pair]
                        stg = (stg0, stg1)[pair]
                        qsl = slice(J * 256, (J + 1) * 256)
                        o_ab = opool.tile([65, 512], F32)
                        pend = None  # software pipeline: PV trails S^T/exp by 1
                        for g in range(J + 1):
                            s_ps = spool.tile([128, 1024], F32)
                            for i, kb in enumerate((2 * g, 2 * g + 1)):
                                for half in range(2):
                                    # concurrent row-group pair must write
                                    # different PSUM banks: slot a bank 0,
                                    # slot b bank 1
                                    seg = half * 512 + i * 256
                                    nc.tensor.matmul(
                                        s_ps[:, seg : seg + 256],
                                        kdr[half * 64 : (half + 1) * 64,
                                            kb * 128 : (kb + 1) * 128],
                                        qt[half * 64 : (half + 1) * 64, qsl],
                                        start=True,
                                        stop=True,
                                    )
                            e_sb = epool.tile([128, 1024], F32R)
                            nc.scalar.activation(
                                e_sb[:], s_ps[:], EXP, bias=0.0, scale=0.125
                            )
                            if g == J:
                                # multiplicative 0/1 causal mask applied after
                                # exp, off the matmul->exp critical path
                                nc.vector.tensor_mul(e_sb[:], e_sb[:], mask_sb[:])
                            if blvl < 2:
                                if pair == 1 and J == NJ - 1 and g == J:
                                    nc.sync.dma_start(
                                        out=out_d[384:512, :],
                                        in_=e_sb[:, 0:HIDDEN].bitcast(F32),
                                    )
                                continue
                            if pend is not None:
                                _emit_pv(nc, o_ab, v_sb, *pend, J)
                            pend = (e_sb, g)
                        if blvl < 2:
                            continue
                        _emit_pv(nc, o_ab, v_sb, *pend, J)
                        if blvl < 3:
                            if J == NJ - 1:
                                oc = rcpool.tile([65, 512], F32, tag="ocdump")
                                nc.vector.tensor_copy(oc[:], o_ab[:])
                                nc.sync.dma_start(
                                    out=out_d[384 + pair * 80 : 449 + pair * 80, 0:512],
                                    in_=oc[:],
                                )
                            continue

                        # normalize: 1/Z, replicate across 64 partitions via
                        # a DRAM-bounce broadcast DMA, then multiply
                        rc = rcpool.tile([128, 512], F32)
                        nc.vector.reciprocal(rc[64:65, :], o_ab[64:65, :])
                        zb = zbpool.tile([1, 512], F32)
                        nc.sync.dma_start(out=zb[:], in_=rc[64:65, :])
                        rz = rcpool.tile([64, 512], F32)
                        nc.sync.dma_start(
                            out=rz[:], in_=zb[0:1, :].broadcast_to([64, 512])
                        )
                        for sl in range(2):
                            csl = slice(sl * 256, (sl + 1) * 256)
                            dst = aoT[0:64, qsl] if sl == 0 else stg[0:64, qsl]
                            nc.vector.tensor_mul(dst, o_ab[0:64, csl], rz[:, csl])
                        # restack this J's slot-b rows into partitions 64..127
                        nc.scalar.dma_start(
                            out=aoT[64:128, qsl], in_=stg[0:64, qsl]
                        )
                    # ---- output projection for this J's two q-blocks,
                    # overlapped with the next J's attention
                    if "C" in phases and blvl >= 3:
                        for qb in (2 * J, 2 * J + 1):
                            f_ps = fpool.tile([128, HIDDEN], F32)
                            for ft in range(2):
                                aoTt = (aoT0, aoT1)[ft]
                                lhsT = aoTt[:, qb * 128 : (qb + 1) * 128]
                                nc.tensor.matmul(
                                    f_ps[:, 0:512],
                                    lhsT,
                                    wo_sb[:, ft * HIDDEN : ft * HIDDEN + 512],
                                    start=(ft == 0),
                                    stop=(ft == 1),
                                )
                                nc.tensor.matmul(
                                    f_ps[:, 512:HIDDEN],
                                    lhsT,
                                    wo_sb[:, ft * HIDDEN + 512 : (ft + 1) * HIDDEN],
                                    start=(ft == 0),
                                    stop=(ft == 1),
                                )
                            ob = obpool.tile([128, HIDDEN], F32)
                            if qb % 2 == 0:
                                nc.vector.tensor_copy(ob[:], f_ps[:])
                            else:
                                nc.scalar.copy(ob[:], f_ps[:])
                            nc.sync.dma_start(
                                out=out_d[qb * 128 : (qb + 1) * 128, :], in_=ob[:]
                            )

            if "B" in phases and "C" not in phases and blvl >= 3:
                nc.sync.dma_start(out=out_d[0:128, :], in_=aoT0[:, 0:HIDDEN].bitcast(F32))
                nc.sync.dma_start(out=out_d[128:256, :], in_=aoT1[:, 0:HIDDEN].bitcast(F32))

            # ===== old phase C: folded into the J loop above =====
            with (
                tc.tile_pool(name="fp2", bufs=1, space="PSUM") as fpool,
                tc.tile_pool(name="ob2", bufs=1) as obpool,
            ):
                for qb in range(0):
                    f_ps = fpool.tile([128, HIDDEN], F32)
                    for ft in range(2):
                        aoT = (aoT0, aoT1)[ft]
                        lhsT = aoT[:, qb * 128 : (qb + 1) * 128]
                        nc.tensor.matmul(
                            f_ps[:, 0:512],
                            lhsT,
                            wo_sb[:, ft * HIDDEN : ft * HIDDEN + 512],
                            start=(ft == 0),
                            stop=(ft == 1),
                        )
                        nc.tensor.matmul(
                            f_ps[:, 512:HIDDEN],
                            lhsT,
                            wo_sb[:, ft * HIDDEN + 512 : (ft + 1) * HIDDEN],
                            start=(ft == 0),
                            stop=(ft == 1),
                        )
                    ob = obpool.tile([128, HIDDEN], F32)
                    if qb % 2 == 0:
                        nc.vector.tensor_copy(ob[:], f_ps[:])
                    else:
                        nc.scalar.copy(ob[:], f_ps[:])
                    nc.sync.dma_start(
                        out=out_d[qb * 128 : (qb + 1) * 128, :], in_=ob[:]
                    )

    nc.compile()
    return nc


def _emit_pv(nc, o_ab, v_sb, e_sb, g, J):
    """PV accumulation for one exp'd group (k-blocks 2g, 2g+1)."""
    for i, kb in enumerate((2 * g, 2 * g + 1)):
        for sl in range(2):
            seg = sl * 512 + i * 256
            # one accumulation group for the whole o_ab tile: start=True
            # clears has_written for the entire PSUM bank, so only the very
            # first matmul may set it
            nc.tensor.matmul(
                o_ab[:, sl * 256 : (sl + 1) * 256],
                v_sb[:, kb * 65 : (kb + 1) * 65],
                e_sb[:, seg : seg + 256],
                start=(g == 0 and i == 0 and sl == 0),
                stop=(g == J and i == 1 and sl == 1),
                skip_group_check=True,
            )


def round_f32r(a):
    """Round-to-nearest-even fp32 -> fp32r (11 mantissa bits)."""
    u = np.ascontiguousarray(a, np.float32).view(np.uint32)
    r = (u + np.uint32(0x7FF) + ((u >> np.uint32(12)) & np.uint32(1))) & np.uint32(
        0xFFFFF000
    )
    return r.view(np.float32)


def _rope_tables():
    inv_freq = 1.0 / (
        ROPE_THETA ** (np.arange(0, HEAD_DIM, 2, dtype=np.float32) / HEAD_DIM)
    )
    t = np.arange(S, dtype=np.float32)
    freqs = np.outer(t, inv_freq)  # [S, 32]
    emb = np.concatenate([freqs, freqs], axis=-1)  # [S, 64]
    cosT = np.cos(emb).T.astype(np.float32)  # [64, S]
    sinT = np.sin(emb).T.astype(np.float32)
    sinmod = sinT.copy()
    sinmod[0:32] = -sinmod[0:32]
    cosd = np.concatenate([cosT, cosT], axis=0)  # [128, S]
    sind = np.concatenate([sinmod, sinmod], axis=0)
    return np.ascontiguousarray(cosd), np.ascontiguousarray(sind)


def _masks():
    kp = np.arange(128)[:, None]
    qp = np.arange(128)[None, :]
    tri = np.where(kp <= qp, 1.0, 0.0).astype(np.float32)  # [128,128]
    ones = np.ones((128, 128), np.float32)
    zeros = np.zeros((128, 128), np.float32)
    mask0 = np.concatenate([tri, ones], axis=1)   # kb 2J vs [2J, 2J+1]
    mask1 = np.concatenate([zeros, tri], axis=1)  # kb 2J+1 vs [2J, 2J+1]
    return np.ascontiguousarray(
        np.concatenate([mask0, mask1, mask0, mask1], axis=1)
    )  # [128, 1024]


def _tile_hsT(hsT):
    """[896, 2048] -> [512, 3584]: row ss*128+p = concat over t of
    hsT[t*128+p, ss*512:(ss+1)*512], matching the SBUF projection layout."""
    out = np.empty((4 * 128, NH7 * 512), np.float32)
    for ss in range(4):
        blk = hsT[:, ss * 512 : (ss + 1) * 512].reshape(NH7, 128, 512)
        out[ss * 128 : (ss + 1) * 128, :] = (
            blk.transpose(1, 0, 2).reshape(128, NH7 * 512)
        )
    return np.ascontiguousarray(out)


_CONST_CACHE = None


def make_in_maps(hidden_states, wq, bq, wk, bk, wv, bv, wo):
    global _CONST_CACHE
    if _CONST_CACHE is None:
        cosd, sind = _rope_tables()
        _CONST_CACHE = (cosd, sind, _masks())
    cosd, sind, maskD = _CONST_CACHE
    # the tiled/rounded hidden states are shared by the 4 cores of a batch
    hs_tiled = [_tile_hsT(round_f32r(hidden_states[b].T)) for b in range(B)]
    in_maps = []
    for core in range(8):
        b, kv, half = core // 4, (core % 4) // 2, core % 2
        if half == 0:
            slots = [kv * 7 + 0, kv * 7 + 1, kv * 7 + 2, kv * 7 + 3]
            dup = []
        else:
            slots = [kv * 7 + 4, kv * 7 + 5, kv * 7 + 6, kv * 7 + 3]
            dup = [3]
        cols = np.concatenate([np.arange(h * 64, (h + 1) * 64) for h in slots])
        wq4 = round_f32r(wq[:, cols])
        bq4 = np.ascontiguousarray(bq[cols].reshape(2, 128))
        wkv = round_f32r(
            np.concatenate(
                [wk[:, kv * 64 : (kv + 1) * 64], wv[:, kv * 64 : (kv + 1) * 64]],
                axis=1,
            )
        )
        bkv = np.ascontiguousarray(
            np.concatenate(
                [bk[kv * 64 : (kv + 1) * 64], bv[kv * 64 : (kv + 1) * 64]]
            ).reshape(1, 128)
        )
        wo4 = wo[cols, :].copy()
        for d in dup:
            wo4[d * 64 : (d + 1) * 64, :] = 0.0
        in_maps.append(
            {
                "hsT": hs_tiled[b],
                "wq4": wq4,
                "bq4": bq4,
                "wkv": wkv,
                "bkv": bkv,
                "wo4": round_f32r(wo4),
                "cosd": cosd,
                "sind": sind,
                "maskD": maskD,
            }
        )
    return in_maps


_NC_CACHE = None


def _get_program():
    global _NC_CACHE
    if _NC_CACHE is None:
        _NC_CACHE = build_program()
    return _NC_CACHE


def kernel(hidden_states, wq, bq, wk, bk, wv, bv, wo):
    hidden_states = np.asarray(hidden_states, np.float32)
    wq = np.asarray(wq, np.float32)
    bq = np.asarray(bq, np.float32)
    wk = np.asarray(wk, np.float32)
    bk = np.asarray(bk, np.float32)
    wv = np.asarray(wv, np.float32)
    bv = np.asarray(bv, np.float32)
    wo = np.asarray(wo, np.float32)

    nc = _get_program()
    in_maps = make_in_maps(hidden_states, wq, bq, wk, bk, wv, bv, wo)
    res = run_bass_kernel_spmd(nc, in_maps, list(range(8)))
    out = np.zeros((B, S, HIDDEN), np.float32)
    for core in range(8):
        out[core // 4] += res.results[core]["out"]
    return out



# revision 3
# speedup vs baseline: 1.3644x; 1.3644x over previous
"""Trainium2 Bass kernel for GQA attention (nn_Attention_40364102648437).

Problem: B=2, S=2048, HIDDEN=896, 14 q heads / 2 kv heads, head_dim 64,
RoPE (theta 1e6), causal softmax, o-projection.

Sharding (8 cores, SPMD): core = b*4 + kv*2 + half.
Each core owns one batch b, one kv head, and 4 q-head slots (7 q heads per
kv group are split 4+3; the last slot of the second half is a duplicate
whose wo rows are zeroed so its contribution vanishes). Every core computes
a full [S, HIDDEN] partial output (its heads' contribution through wo);
the host sums the 4 partials per batch.

Engine budget (cost model): PE does all matmuls (~92us at 2.4GHz), the
Activation engine does ONLY the softmax exp (72 x [128,1024] ~= 75us), DVE
does RoPE/mask/bias/normalize element-wise work in bf16 (2-byte dtypes get
2-4x DVE rate), Pool (gpsimd) does PSUM->SBUF copies and the softmax
1/Z partition-broadcast, SP issues every DMA. The whole attention path is
bf16 (same PE rate as fp32r, half the DMA bytes, no fp32r-producer rule).

Softmax normalization: V tiles carry a ones column so PV accumulates the
denominator Z in o_ab row 64; 1/Z is computed by DVE reciprocal into a
[1,512] SBUF row, replicated across partitions with gpsimd
partition_broadcast (no DRAM bounce), and multiplied into the attention
output as it is copied to SBUF (bf16) for the o-projection.

Pipelining: PV trails scores/exp by one k-group; the o-projection of
superblock J is emitted after attention of J+1's first pair so the PE
never waits on the slot-b restack DMA; output rows DMA straight from an
SBUF staging tile.

Hardware constraints (from the previous session, kept intact):
  - concurrent row-group matmuls (partition bases 0/64) must write
    different PSUM banks -> s_ps puts slot a in bank 0, slot b in bank 1;
  - matmul start=True clears has_written for its PSUM region, so o_ab
    gets exactly one start/stop accumulation group;
  - engines cannot move data across partitions: RoPE's rotate-half swap,
    the k row duplication, and the slot-b restack use SBUF->SBUF DMA;
  - tensor_tensor may read only one input from PSUM.
"""

import numpy as np
import ml_dtypes

import concourse.bass as bass
import concourse.mybir as mybir
from concourse import bacc
from concourse.tile import TileContext
from concourse.masks import make_identity
from concourse.bass_utils import run_bass_kernel_spmd

F32 = mybir.dt.float32
BF16 = mybir.dt.bfloat16
BF = ml_dtypes.bfloat16

HIDDEN = 896
HEAD_DIM = 64
B = 2
S = 2048
ROPE_THETA = 1000000.0
NH7 = HIDDEN // 128  # 7 hidden tiles
NKB = S // 128       # 16 key blocks
NJ = S // 256        # 8 query superblocks (256 q positions each)


def build_program():
    nc = bacc.Bacc("TRN2", target_bir_lowering=False, debug=False, num_devices=8)

    # host-pre-tiled: row ss*128+p holds [t, n] -> hs[b][ss*512+n, t*128+p]
    hsT = nc.dram_tensor("hsT", [4 * 128, NH7 * 512], BF16, kind="ExternalInput")
    wq4 = nc.dram_tensor("wq4", [HIDDEN, 256], BF16, kind="ExternalInput")
    bq4 = nc.dram_tensor("bq4", [2, 128], F32, kind="ExternalInput")
    wkv = nc.dram_tensor("wkv", [HIDDEN, 128], BF16, kind="ExternalInput")
    bkv = nc.dram_tensor("bkv", [1, 128], F32, kind="ExternalInput")
    wo4 = nc.dram_tensor("wo4", [256, HIDDEN], BF16, kind="ExternalInput")
    cosd = nc.dram_tensor("cosd", [128, S], BF16, kind="ExternalInput")
    sind = nc.dram_tensor("sind", [128, S], BF16, kind="ExternalInput")
    maskD = nc.dram_tensor("maskD", [128, 1024], BF16, kind="ExternalInput")
    out_d = nc.dram_tensor("out", [S, HIDDEN], F32, kind="ExternalOutput")

    EXP = mybir.ActivationFunctionType.Exp

    with TileContext(nc) as tc:
        with (
            tc.tile_pool(name="const", bufs=1) as cpool,
            tc.tile_pool(name="big", bufs=1) as bigpool,
        ):
            # ---- constants (one batched DMA each)
            wkv_sb = cpool.tile([128, NH7 * 128], BF16)
            nc.sync.dma_start(
                out=wkv_sb[:].rearrange("p (t f) -> p t f", t=NH7),
                in_=wkv.rearrange("(t p) f -> p t f", p=128),
            )
            wq_sb = cpool.tile([128, NH7 * 256], BF16)
            nc.sync.dma_start(
                out=wq_sb[:].rearrange("p (t f) -> p t f", t=NH7),
                in_=wq4.rearrange("(t p) f -> p t f", p=128),
            )
            wo_sb = cpool.tile([128, 2 * HIDDEN], BF16)
            nc.sync.dma_start(
                out=wo_sb[:].rearrange("p (t f) -> p t f", t=2),
                in_=wo4.rearrange("(t p) f -> p t f", p=128),
            )
            cos_sb = cpool.tile([128, S], BF16)
            nc.sync.dma_start(out=cos_sb[:], in_=cosd[:])
            sin_sb = cpool.tile([128, S], BF16)
            nc.sync.dma_start(out=sin_sb[:], in_=sind[:])
            mask_sb = cpool.tile([128, 1024], BF16)
            nc.sync.dma_start(out=mask_sb[:], in_=maskD[:])
            bq_sb = cpool.tile([128, 2], F32)
            nc.sync.dma_start(out=bq_sb[:], in_=bq4.rearrange("a p -> p a"))
            bkv_sb = cpool.tile([128, 1], F32)
            nc.sync.dma_start(out=bkv_sb[:], in_=bkv.rearrange("a p -> p a"))
            identb = cpool.tile([128, 128], BF16)
            make_identity(nc, identb[:])

            # ---- persistent activations (all bf16)
            kvT = bigpool.tile([128, S], BF16)   # rows 0-63 k, 64-127 vT
            kdr = bigpool.tile([128, S], BF16)   # rope'd k, duplicated halves
            qA = bigpool.tile([128, S], BF16)
            qB = bigpool.tile([128, S], BF16)
            qAr = bigpool.tile([128, S], BF16)
            qBr = bigpool.tile([128, S], BF16)
            v_sb = bigpool.tile([128, NKB * 65], BF16)
            aoT0 = bigpool.tile([128, S], BF16)
            aoT1 = bigpool.tile([128, S], BF16)
            stg0 = bigpool.tile([64, S], BF16)
            stg1 = bigpool.tile([64, S], BF16)

            # ================= phase A: projections =================
            with (
                tc.tile_pool(name="hst", bufs=4) as hpool,
                tc.tile_pool(name="pps", bufs=2, space="PSUM") as ppool,
                tc.tile_pool(name="swp", bufs=3) as swpool,
                tc.tile_pool(name="vtr", bufs=2, space="PSUM") as vpool,
            ):
                hs_tiles = []
                for ss in range(4):
                    ssl = slice(ss * 512, (ss + 1) * 512)
                    hs_t = hpool.tile([128, NH7 * 512], BF16)
                    hs_tiles.append(hs_t)
                    nc.sync.dma_start(
                        out=hs_t[:], in_=hsT[ss * 128 : (ss + 1) * 128, :]
                    )
                    kv_ps = ppool.tile([128, 512], F32)
                    for h in range(NH7):
                        nc.tensor.matmul(
                            kv_ps[:],
                            wkv_sb[:, h * 128 : (h + 1) * 128],
                            hs_t[:, h * 512 : (h + 1) * 512],
                            start=(h == 0),
                            stop=(h == NH7 - 1),
                        )
                    nc.vector.tensor_scalar_add(kvT[:, ssl], kv_ps[:], bkv_sb[:, 0:1])

                # ---- RoPE on k (rows 0-63 of kvT), then duplicate halves
                ktw = swpool.tile([64, S], BF16)
                nc.sync.dma_start(out=ktw[0:32, :], in_=kvT[32:64, :])
                nc.sync.dma_start(out=ktw[32:64, :], in_=kvT[0:32, :])
                nc.vector.tensor_mul(ktw[:], ktw[:], sin_sb[0:64, :])
                nc.vector.tensor_mul(kvT[0:64, :], kvT[0:64, :], cos_sb[0:64, :])
                nc.vector.tensor_add(kdr[0:64, :], kvT[0:64, :], ktw[:])
                nc.sync.dma_start(out=kdr[64:128, :], in_=kdr[0:64, :])

                # ---- v natural layout [k_pos, 64] + ones column (col 64)
                for kb in range(NKB):
                    vt_ps = vpool.tile([128, 64], BF16)
                    nc.tensor.transpose(
                        vt_ps[:],
                        kvT[64:128, kb * 128 : (kb + 1) * 128],
                        identb[64:128, 64:128],
                    )
                    nc.gpsimd.tensor_copy(v_sb[:, kb * 65 : kb * 65 + 64], vt_ps[:])
                ones_ap = v_sb[:, 64 : NKB * 65 : 65]
                nc.gpsimd.memset(ones_ap, 1.0)

                # ---- q projections (hs tiles stay resident) + RoPE
                for ss in range(4):
                    ssl = slice(ss * 512, (ss + 1) * 512)
                    hs_t = hs_tiles[ss]
                    for ft in range(2):
                        q_ps = ppool.tile([128, 512], F32)
                        for h in range(NH7):
                            nc.tensor.matmul(
                                q_ps[:],
                                wq_sb[:, h * 256 + ft * 128 : h * 256 + (ft + 1) * 128],
                                hs_t[:, h * 512 : (h + 1) * 512],
                                start=(h == 0),
                                stop=(h == NH7 - 1),
                            )
                        qt = (qA, qB)[ft]
                        nc.vector.tensor_scalar_add(
                            qt[:, ssl], q_ps[:], bq_sb[:, ft : ft + 1]
                        )
                for t, tr in ((qA, qAr), (qB, qBr)):
                    tsw = swpool.tile([128, S], BF16)
                    for dst, src in ((0, 32), (32, 0), (64, 96), (96, 64)):
                        nc.sync.dma_start(
                            out=tsw[dst : dst + 32, :], in_=t[src : src + 32, :]
                        )
                    nc.vector.tensor_mul(tsw[:], tsw[:], sin_sb[:])
                    nc.vector.tensor_mul(t[:], t[:], cos_sb[:])
                    nc.vector.tensor_add(tr[:], t[:], tsw[:])

            # ================= phase B: attention + o-projection =================
            with (
                tc.tile_pool(name="sps", bufs=2, space="PSUM") as spool,
                tc.tile_pool(name="ops", bufs=2, space="PSUM") as opool,
                tc.tile_pool(name="fps", bufs=2, space="PSUM") as fpool,
                tc.tile_pool(name="esb", bufs=4) as epool,
                tc.tile_pool(name="rzs", bufs=2) as rzpool,
                tc.tile_pool(name="bcs", bufs=2) as bcpool,
                tc.tile_pool(name="osb", bufs=3) as obpool,
            ):
                def emit_oproj(J):
                    for qb in (2 * J, 2 * J + 1):
                        ob = obpool.tile([128, HIDDEN], F32)
                        for half in range(2):
                            hsl = slice(half * 448, (half + 1) * 448)
                            f_ps = fpool.tile([128, 448], F32)
                            for ft in range(2):
                                aoTt = (aoT0, aoT1)[ft]
                                nc.tensor.matmul(
                                    f_ps[:],
                                    aoTt[:, qb * 128 : (qb + 1) * 128],
                                    wo_sb[:, ft * HIDDEN + half * 448 :
                                          ft * HIDDEN + (half + 1) * 448],
                                    start=(ft == 0),
                                    stop=(ft == 1),
                                )
                            nc.gpsimd.tensor_copy(ob[:, hsl], f_ps[:])
                        nc.sync.dma_start(
                            out=out_d[qb * 128 : (qb + 1) * 128, :], in_=ob[:]
                        )

                for J in range(NJ):
                    for pair in range(2):
                        qt = (qAr, qBr)[pair]
                        aoT = (aoT0, aoT1)[pair]
                        stg = (stg0, stg1)[pair]
                        qsl = slice(J * 256, (J + 1) * 256)
                        o_ab = opool.tile([65, 512], F32)
                        pend = None  # software pipeline: PV trails S^T/exp by 1
                        for g in range(J + 1):
                            s_ps = spool.tile([128, 1024], F32)
                            for i, kb in enumerate((2 * g, 2 * g + 1)):
                                for half in range(2):
                                    # concurrent row-group pair must write
                                    # different PSUM banks: slot a bank 0,
                                    # slot b bank 1
                                    seg = half * 512 + i * 256
                                    nc.tensor.matmul(
                                        s_ps[:, seg : seg + 256],
                                        kdr[half * 64 : (half + 1) * 64,
                                            kb * 128 : (kb + 1) * 128],
                                        qt[half * 64 : (half + 1) * 64, qsl],
                                        start=True,
                                        stop=True,
                                    )
                            e_sb = epool.tile([128, 1024], BF16)
                            nc.scalar.activation(
                                e_sb[:], s_ps[:], EXP, bias=0.0, scale=0.125
                            )
                            if g == J:
                                # multiplicative 0/1 causal mask after exp
                                # (bf16, all-SBUF -> 4x DVE rate)
                                nc.vector.tensor_mul(e_sb[:], e_sb[:], mask_sb[:])
                            if pend is not None:
                                _emit_pv(nc, o_ab, v_sb, *pend, J)
                            pend = (e_sb, g)
                        _emit_pv(nc, o_ab, v_sb, *pend, J)

                        # normalize: 1/Z from o_ab row 64, replicated across
                        # partitions on the Pool engine, multiplied in as the
                        # attention output is copied to SBUF (bf16)
                        rz = rzpool.tile([1, 512], F32)
                        nc.vector.reciprocal(rz[:], o_ab[64:65, :])
                        bc = bcpool.tile([64, 512], F32)
                        nc.gpsimd.partition_broadcast(bc[:], rz[0:1, :])
                        nc.vector.tensor_mul(
                            aoT[0:64, qsl], o_ab[0:64, 0:256], bc[:, 0:256]
                        )
                        nc.vector.tensor_mul(
                            stg[0:64, qsl], o_ab[0:64, 256:512], bc[:, 256:512]
                        )
                        # restack slot-b rows into partitions 64..127
                        nc.sync.dma_start(out=aoT[64:128, qsl], in_=stg[0:64, qsl])
                    # o-projection deferred one J so the PE never waits on
                    # the restack DMA chain
                    if J > 0:
                        emit_oproj(J - 1)
                emit_oproj(NJ - 1)

    nc.compile()
    return nc


def _emit_pv(nc, o_ab, v_sb, e_sb, g, J):
    """PV accumulation for one exp'd group (k-blocks 2g, 2g+1)."""
    for i, kb in enumerate((2 * g, 2 * g + 1)):
        for sl in range(2):
            seg = sl * 512 + i * 256
            # one accumulation group for the whole o_ab tile: start=True
            # clears has_written for the entire PSUM bank, so only the very
            # first matmul may set it
            nc.tensor.matmul(
                o_ab[:, sl * 256 : (sl + 1) * 256],
                v_sb[:, kb * 65 : (kb + 1) * 65],
                e_sb[:, seg : seg + 256],
                start=(g == 0 and i == 0 and sl == 0),
                stop=(g == J and i == 1 and sl == 1),
                skip_group_check=True,
            )


def _rope_tables():
    inv_freq = 1.0 / (
        ROPE_THETA ** (np.arange(0, HEAD_DIM, 2, dtype=np.float32) / HEAD_DIM)
    )
    t = np.arange(S, dtype=np.float32)
    freqs = np.outer(t, inv_freq)  # [S, 32]
    emb = np.concatenate([freqs, freqs], axis=-1)  # [S, 64]
    cosT = np.cos(emb).T.astype(np.float32)  # [64, S]
    sinT = np.sin(emb).T.astype(np.float32)
    sinmod = sinT.copy()
    sinmod[0:32] = -sinmod[0:32]
    cosd = np.concatenate([cosT, cosT], axis=0)  # [128, S]
    sind = np.concatenate([sinmod, sinmod], axis=0)
    return np.ascontiguousarray(cosd.astype(BF)), np.ascontiguousarray(
        sind.astype(BF)
    )


def _masks():
    kp = np.arange(128)[:, None]
    qp = np.arange(128)[None, :]
    tri = np.where(kp <= qp, 1.0, 0.0).astype(np.float32)  # [128,128]
    ones = np.ones((128, 128), np.float32)
    zeros = np.zeros((128, 128), np.float32)
    mask0 = np.concatenate([tri, ones], axis=1)   # kb 2J vs [2J, 2J+1]
    mask1 = np.concatenate([zeros, tri], axis=1)  # kb 2J+1 vs [2J, 2J+1]
    return np.ascontiguousarray(
        np.concatenate([mask0, mask1, mask0, mask1], axis=1).astype(BF)
    )  # [128, 1024]


def _tile_hsT(hsT):
    """[896, 2048] -> [512, 3584]: row ss*128+p = concat over t of
    hsT[t*128+p, ss*512:(ss+1)*512], matching the SBUF projection layout."""
    out = np.empty((4 * 128, NH7 * 512), BF)
    for ss in range(4):
        blk = hsT[:, ss * 512 : (ss + 1) * 512].reshape(NH7, 128, 512)
        out[ss * 128 : (ss + 1) * 128, :] = (
            blk.transpose(1, 0, 2).reshape(128, NH7 * 512).astype(BF)
        )
    return np.ascontiguousarray(out)


_CONST_CACHE = None


def make_in_maps(hidden_states, wq, bq, wk, bk, wv, bv, wo):
    global _CONST_CACHE
    if _CONST_CACHE is None:
        cosd, sind = _rope_tables()
        _CONST_CACHE = (cosd, sind, _masks())
    cosd, sind, maskD = _CONST_CACHE
    # the tiled hidden states are shared by the 4 cores of a batch
    hs_tiled = [_tile_hsT(hidden_states[b].T) for b in range(B)]
    in_maps = []
    for core in range(8):
        b, kv, half = core // 4, (core % 4) // 2, core % 2
        if half == 0:
            slots = [kv * 7 + 0, kv * 7 + 1, kv * 7 + 2, kv * 7 + 3]
            dup = []
        else:
            slots = [kv * 7 + 4, kv * 7 + 5, kv * 7 + 6, kv * 7 + 3]
            dup = [3]
        cols = np.concatenate([np.arange(h * 64, (h + 1) * 64) for h in slots])
        wq4 = np.ascontiguousarray(wq[:, cols].astype(BF))
        bq4 = np.ascontiguousarray(bq[cols].reshape(2, 128))
        wkv = np.ascontiguousarray(
            np.concatenate(
                [wk[:, kv * 64 : (kv + 1) * 64], wv[:, kv * 64 : (kv + 1) * 64]],
                axis=1,
            ).astype(BF)
        )
        bkv = np.ascontiguousarray(
            np.concatenate(
                [bk[kv * 64 : (kv + 1) * 64], bv[kv * 64 : (kv + 1) * 64]]
            ).reshape(1, 128)
        )
        wo4 = wo[cols, :].copy()
        for d in dup:
            wo4[d * 64 : (d + 1) * 64, :] = 0.0
        in_maps.append(
            {
                "hsT": hs_tiled[b],
                "wq4": wq4,
                "bq4": bq4,
                "wkv": wkv,
                "bkv": bkv,
                "wo4": np.ascontiguousarray(wo4.astype(BF)),
                "cosd": cosd,
                "sind": sind,
                "maskD": maskD,
            }
        )
    return in_maps


_NC_CACHE = None


def _get_program():
    global _NC_CACHE
    if _NC_CACHE is None:
        _NC_CACHE = build_program()
    return _NC_CACHE


def kernel(hidden_states, wq, bq, wk, bk, wv, bv, wo):
    hidden_states = np.asarray(hidden_states, np.float32)
    wq = np.asarray(wq, np.float32)
    bq = np.asarray(bq, np.float32)
    wk = np.asarray(wk, np.float32)
    bk = np.asarray(bk, np.float32)
    wv = np.asarray(wv, np.float32)
    bv = np.asarray(bv, np.float32)
    wo = np.asarray(wo, np.float32)

    nc = _get_program()
    in_maps = make_in_maps(hidden_states, wq, bq, wk, bk, wv, bv, wo)
    res = run_bass_kernel_spmd(nc, in_maps, list(range(8)))
    out = np.zeros((B, S, HIDDEN), np.float32)
    for core in range(8):
        out[core // 4] += res.results[core]["out"]
    return out


# revision 9
# speedup vs baseline: 1.4034x; 1.0286x over previous
"""Trainium2 Bass kernel for GQA attention (nn_Attention_40364102648437).

Problem: B=2, S=2048, HIDDEN=896, 14 q heads / 2 kv heads, head_dim 64,
RoPE (theta 1e6), causal softmax, o-projection.

Sharding (8 cores, SPMD): core = b*4 + kv*2 + half.
Each core owns one batch b, one kv head, and 4 q-head slots (7 q heads per
kv group are split 4+3; the last slot of the second half is a duplicate
whose wo rows are zeroed so its contribution vanishes). Every core computes
a full [S, HIDDEN] partial output (its heads' contribution through wo);
the host sums the 4 partials per batch.

Engine budget (cost model): PE does all matmuls (~92us at 2.4GHz), the
Activation engine does ONLY the softmax exp (72 x [128,1024] ~= 75us), DVE
does RoPE/mask/bias/normalize element-wise work in bf16 (2-byte dtypes get
2-4x DVE rate), Pool (gpsimd) does PSUM->SBUF copies and the softmax
1/Z partition-broadcast, SP issues every DMA. The whole attention path is
bf16 (same PE rate as fp32r, half the DMA bytes, no fp32r-producer rule).

Softmax normalization: V tiles carry a ones column so PV accumulates the
denominator Z in o_ab row 64; 1/Z is computed by DVE reciprocal into a
[1,512] SBUF row, replicated across partitions with gpsimd
partition_broadcast (no DRAM bounce), and multiplied into the attention
output as it is copied to SBUF (bf16) for the o-projection.

Pipelining: PV trails scores/exp by one k-group; the o-projection of
superblock J is emitted after attention of J+1's first pair so the PE
never waits on the slot-b restack DMA; output rows DMA straight from an
SBUF staging tile.

Hardware constraints (from the previous session, kept intact):
  - concurrent row-group matmuls (partition bases 0/64) must write
    different PSUM banks -> s_ps puts slot a in bank 0, slot b in bank 1;
  - matmul start=True clears has_written for its PSUM region, so o_ab
    gets exactly one start/stop accumulation group;
  - engines cannot move data across partitions: RoPE's rotate-half swap,
    the k row duplication, and the slot-b restack use SBUF->SBUF DMA;
  - tensor_tensor may read only one input from PSUM.
"""

import numpy as np
import ml_dtypes

import concourse.bass as bass
import concourse.mybir as mybir
from concourse import bacc
from concourse.tile import TileContext
from concourse.masks import make_identity
from concourse.bass_utils import run_bass_kernel_spmd

F32 = mybir.dt.float32
BF16 = mybir.dt.bfloat16
BF = ml_dtypes.bfloat16

HIDDEN = 896
HEAD_DIM = 64
B = 2
S = 2048
ROPE_THETA = 1000000.0
NH7 = HIDDEN // 128  # 7 hidden tiles
NKB = S // 128       # 16 key blocks
NJ = S // 256        # 8 query superblocks (256 q positions each)


def build_program():
    nc = bacc.Bacc("TRN2", target_bir_lowering=False, debug=False, num_devices=8)

    # host-pre-tiled: row ss*128+p holds [t, n] -> hs[b][ss*512+n, t*128+p]
    hsT = nc.dram_tensor("hsT", [4 * 128, NH7 * 512], BF16, kind="ExternalInput")
    wq4 = nc.dram_tensor("wq4", [HIDDEN, 256], BF16, kind="ExternalInput")
    bq4 = nc.dram_tensor("bq4", [2, 128], F32, kind="ExternalInput")
    wkv = nc.dram_tensor("wkv", [HIDDEN, 128], BF16, kind="ExternalInput")
    bkv = nc.dram_tensor("bkv", [1, 128], F32, kind="ExternalInput")
    wo4 = nc.dram_tensor("wo4", [256, HIDDEN], BF16, kind="ExternalInput")
    cosd = nc.dram_tensor("cosd", [128, S], BF16, kind="ExternalInput")
    sind = nc.dram_tensor("sind", [128, S], BF16, kind="ExternalInput")
    maskD = nc.dram_tensor("maskD", [128, 1024], BF16, kind="ExternalInput")
    out_d = nc.dram_tensor("out", [S, HIDDEN], F32, kind="ExternalOutput")

    EXP = mybir.ActivationFunctionType.Exp

    with TileContext(nc) as tc:
        with (
            tc.tile_pool(name="const", bufs=1) as cpool,
            tc.tile_pool(name="big", bufs=1) as bigpool,
        ):
            # ---- constants, issued in first-use order (DMAs serialize on
            # the single HWDGE device at ~625ns each)
            wkv_sb = cpool.tile([128, NH7 * 128], BF16)
            nc.sync.dma_start(
                out=wkv_sb[:].rearrange("p (t f) -> p t f", t=NH7),
                in_=wkv.rearrange("(t p) f -> p t f", p=128),
            )
            bkv_sb = cpool.tile([128, 1], F32)
            nc.sync.dma_start(out=bkv_sb[:], in_=bkv.rearrange("a p -> p a"))
            wq_sb = cpool.tile([128, NH7 * 256], BF16)
            bq_sb = cpool.tile([128, 2], F32)
            cos_sb = cpool.tile([128, S], BF16)
            sin_sb = cpool.tile([128, S], BF16)
            wo_sb = cpool.tile([128, 2 * HIDDEN], BF16)
            mask_sb = cpool.tile([128, 1024], BF16)
            identb = cpool.tile([128, 128], BF16)
            make_identity(nc, identb[:])

            def load_consts():
                nc.sync.dma_start(out=cos_sb[:], in_=cosd[:])
                nc.sync.dma_start(out=sin_sb[:], in_=sind[:])
                nc.sync.dma_start(
                    out=wq_sb[:].rearrange("p (t f) -> p t f", t=NH7),
                    in_=wq4.rearrange("(t p) f -> p t f", p=128),
                )
                nc.sync.dma_start(out=bq_sb[:], in_=bq4.rearrange("a p -> p a"))
                nc.sync.dma_start(
                    out=wo_sb[:].rearrange("p (t f) -> p t f", t=2),
                    in_=wo4.rearrange("(t p) f -> p t f", p=128),
                )
                nc.sync.dma_start(out=mask_sb[:], in_=maskD[:])

            # ---- persistent activations (all bf16)
            kvT = bigpool.tile([128, S], BF16)   # rows 0-63 k, 64-127 vT
            kdr = bigpool.tile([128, S], BF16)   # rope'd k, duplicated halves
            qA = bigpool.tile([128, S], BF16)
            qB = bigpool.tile([128, S], BF16)
            qAr = bigpool.tile([128, S], BF16)
            qBr = bigpool.tile([128, S], BF16)
            v_sb = bigpool.tile([128, NKB * 65], BF16)
            aoT0 = bigpool.tile([128, S], BF16)
            aoT1 = bigpool.tile([128, S], BF16)
            stg0 = bigpool.tile([64, S], BF16)
            stg1 = bigpool.tile([64, S], BF16)

            # ================= phase A: projections =================
            with (
                tc.tile_pool(name="hst", bufs=4) as hpool,
                tc.tile_pool(name="pps", bufs=2, space="PSUM") as ppool,
                tc.tile_pool(name="swp", bufs=3) as swpool,
                tc.tile_pool(name="vtr", bufs=2, space="PSUM") as vpool,
            ):
                hs_tiles = []
                for ss in range(4):
                    ssl = slice(ss * 512, (ss + 1) * 512)
                    hs_t = hpool.tile([128, NH7 * 512], BF16)
                    hs_tiles.append(hs_t)
                    if ss == 0:
                        # split so the first kv matmuls start sooner
                        nc.sync.dma_start(
                            out=hs_t[:, 0 : 3 * 512],
                            in_=hsT[0:128, 0 : 3 * 512],
                        )
                        nc.sync.dma_start(
                            out=hs_t[:, 3 * 512 :],
                            in_=hsT[0:128, 3 * 512 :],
                        )
                        load_consts()
                    else:
                        nc.sync.dma_start(
                            out=hs_t[:], in_=hsT[ss * 128 : (ss + 1) * 128, :]
                        )
                    kv_ps = ppool.tile([128, 512], F32)
                    for h in range(NH7):
                        nc.tensor.matmul(
                            kv_ps[:],
                            wkv_sb[:, h * 128 : (h + 1) * 128],
                            hs_t[:, h * 512 : (h + 1) * 512],
                            start=(h == 0),
                            stop=(h == NH7 - 1),
                        )
                    nc.vector.tensor_scalar_add(kvT[:, ssl], kv_ps[:], bkv_sb[:, 0:1])

                # ---- RoPE on k (rows 0-63 of kvT), then duplicate halves.
                # DVE/DMA only: the PE proceeds straight into the q
                # projections, hiding the rope latency.
                ktw = swpool.tile([64, S], BF16)
                nc.sync.dma_start(out=ktw[0:32, :], in_=kvT[32:64, :])
                nc.sync.dma_start(out=ktw[32:64, :], in_=kvT[0:32, :])
                nc.vector.tensor_mul(ktw[:], ktw[:], sin_sb[0:64, :])
                nc.vector.tensor_mul(kvT[0:64, :], kvT[0:64, :], cos_sb[0:64, :])
                nc.vector.tensor_add(kdr[0:64, :], kvT[0:64, :], ktw[:])
                nc.sync.dma_start(out=kdr[64:128, :], in_=kdr[0:64, :])

                # ---- q projections (hs tiles stay resident) + RoPE
                for ss in range(4):
                    ssl = slice(ss * 512, (ss + 1) * 512)
                    hs_t = hs_tiles[ss]
                    for ft in range(2):
                        q_ps = ppool.tile([128, 512], F32)
                        for h in range(NH7):
                            nc.tensor.matmul(
                                q_ps[:],
                                wq_sb[:, h * 256 + ft * 128 : h * 256 + (ft + 1) * 128],
                                hs_t[:, h * 512 : (h + 1) * 512],
                                start=(h == 0),
                                stop=(h == NH7 - 1),
                            )
                        qt = (qA, qB)[ft]
                        nc.vector.tensor_scalar_add(
                            qt[:, ssl], q_ps[:], bq_sb[:, ft : ft + 1]
                        )
                for t, tr in ((qA, qAr), (qB, qBr)):
                    tsw = swpool.tile([128, S], BF16)
                    for dst, src in ((0, 32), (32, 0), (64, 96), (96, 64)):
                        nc.sync.dma_start(
                            out=tsw[dst : dst + 32, :], in_=t[src : src + 32, :]
                        )
                    nc.vector.tensor_mul(tsw[:], tsw[:], sin_sb[:])
                    nc.vector.tensor_mul(t[:], t[:], cos_sb[:])
                    nc.vector.tensor_add(tr[:], t[:], tsw[:])

                # ---- v natural layout [k_pos, 64] + ones column (col 64)
                for kb in range(NKB):
                    vt_ps = vpool.tile([128, 64], BF16)
                    nc.tensor.transpose(
                        vt_ps[:],
                        kvT[64:128, kb * 128 : (kb + 1) * 128],
                        identb[64:128, 64:128],
                    )
                    nc.gpsimd.tensor_copy(v_sb[:, kb * 65 : kb * 65 + 64], vt_ps[:])
                ones_ap = v_sb[:, 64 : NKB * 65 : 65]
                nc.gpsimd.memset(ones_ap, 1.0)

            # ================= phase B: attention + o-projection =================
            with (
                tc.tile_pool(name="sps", bufs=2, space="PSUM") as spool,
                tc.tile_pool(name="ops", bufs=2, space="PSUM") as opool,
                tc.tile_pool(name="fps", bufs=2, space="PSUM") as fpool,
                tc.tile_pool(name="esb", bufs=4) as epool,
                tc.tile_pool(name="rzs", bufs=2) as rzpool,
                tc.tile_pool(name="bcs", bufs=2) as bcpool,
                tc.tile_pool(name="osb", bufs=3) as obpool,
            ):
                def emit_oproj(J):
                    for qb in (2 * J, 2 * J + 1):
                        ob = obpool.tile([128, HIDDEN], F32)
                        for half in range(2):
                            hsl = slice(half * 448, (half + 1) * 448)
                            f_ps = fpool.tile([128, 448], F32)
                            for ft in range(2):
                                aoTt = (aoT0, aoT1)[ft]
                                nc.tensor.matmul(
                                    f_ps[:],
                                    aoTt[:, qb * 128 : (qb + 1) * 128],
                                    wo_sb[:, ft * HIDDEN + half * 448 :
                                          ft * HIDDEN + (half + 1) * 448],
                                    start=(ft == 0),
                                    stop=(ft == 1),
                                )
                            nc.gpsimd.tensor_copy(ob[:, hsl], f_ps[:])
                        nc.sync.dma_start(
                            out=out_d[qb * 128 : (qb + 1) * 128, :], in_=ob[:]
                        )

                for J in range(NJ):
                    for pair in range(2):
                        qt = (qAr, qBr)[pair]
                        aoT = (aoT0, aoT1)[pair]
                        stg = (stg0, stg1)[pair]
                        qsl = slice(J * 256, (J + 1) * 256)
                        o_ab = opool.tile([65, 512], F32)
                        pend = None  # software pipeline: PV trails S^T/exp by 1
                        # diagonal group first: its exp -> mask -> PV chain
                        # pipelines like any other group instead of stalling
                        # the PE at the end of the pair
                        gorder = [J] + list(range(J))
                        for gi, g in enumerate(gorder):
                            s_ps = spool.tile([128, 1024], F32)
                            for i, kb in enumerate((2 * g, 2 * g + 1)):
                                for half in range(2):
                                    # concurrent row-group pair must write
                                    # different PSUM banks: slot a bank 0,
                                    # slot b bank 1
                                    seg = half * 512 + i * 256
                                    nc.tensor.matmul(
                                        s_ps[:, seg : seg + 256],
                                        kdr[half * 64 : (half + 1) * 64,
                                            kb * 128 : (kb + 1) * 128],
                                        qt[half * 64 : (half + 1) * 64, qsl],
                                        start=True,
                                        stop=True,
                                    )
                            e_sb = epool.tile([128, 1024], BF16)
                            nc.scalar.activation(
                                e_sb[:], s_ps[:], EXP, bias=0.0, scale=0.125
                            )
                            if g == J:
                                # multiplicative 0/1 causal mask after exp
                                # (bf16, all-SBUF -> 4x DVE rate)
                                nc.vector.tensor_mul(e_sb[:], e_sb[:], mask_sb[:])
                            if pend is not None:
                                _emit_pv(nc, o_ab, v_sb, *pend)
                            pend = (e_sb, g, gi == 0, gi == J)
                        _emit_pv(nc, o_ab, v_sb, *pend)

                        # normalize: 1/Z from o_ab row 64, replicated across
                        # partitions on the Pool engine, multiplied in as the
                        # attention output is copied to SBUF (bf16)
                        rz = rzpool.tile([1, 512], F32)
                        nc.vector.reciprocal(rz[:], o_ab[64:65, :])
                        bc = bcpool.tile([64, 512], F32)
                        nc.gpsimd.partition_broadcast(bc[:], rz[0:1, :])
                        nc.vector.tensor_mul(
                            aoT[0:64, qsl], o_ab[0:64, 0:256], bc[:, 0:256]
                        )
                        nc.vector.tensor_mul(
                            stg[0:64, qsl], o_ab[0:64, 256:512], bc[:, 256:512]
                        )
                        # restack slot-b rows into partitions 64..127
                        nc.sync.dma_start(out=aoT[64:128, qsl], in_=stg[0:64, qsl])
                    # o-projection deferred one J so the PE never waits on
                    # the restack DMA chain
                    if J > 0:
                        emit_oproj(J - 1)
                emit_oproj(NJ - 1)

    nc.compile()
    return nc


def _emit_pv(nc, o_ab, v_sb, e_sb, g, first, last):
    """PV accumulation for one exp'd group (k-blocks 2g, 2g+1)."""
    for i, kb in enumerate((2 * g, 2 * g + 1)):
        for sl in range(2):
            seg = sl * 512 + i * 256
            # one accumulation group for the whole o_ab tile: start=True
            # clears has_written for the entire PSUM bank, so only the very
            # first matmul may set it
            nc.tensor.matmul(
                o_ab[:, sl * 256 : (sl + 1) * 256],
                v_sb[:, kb * 65 : (kb + 1) * 65],
                e_sb[:, seg : seg + 256],
                start=(first and i == 0 and sl == 0),
                stop=(last and i == 1 and sl == 1),
                skip_group_check=True,
            )


def _rope_tables():
    inv_freq = 1.0 / (
        ROPE_THETA ** (np.arange(0, HEAD_DIM, 2, dtype=np.float32) / HEAD_DIM)
    )
    t = np.arange(S, dtype=np.float32)
    freqs = np.outer(t, inv_freq)  # [S, 32]
    emb = np.concatenate([freqs, freqs], axis=-1)  # [S, 64]
    cosT = np.cos(emb).T.astype(np.float32)  # [64, S]
    sinT = np.sin(emb).T.astype(np.float32)
    sinmod = sinT.copy()
    sinmod[0:32] = -sinmod[0:32]
    cosd = np.concatenate([cosT, cosT], axis=0)  # [128, S]
    sind = np.concatenate([sinmod, sinmod], axis=0)
    return np.ascontiguousarray(cosd.astype(BF)), np.ascontiguousarray(
        sind.astype(BF)
    )


def _masks():
    kp = np.arange(128)[:, None]
    qp = np.arange(128)[None, :]
    tri = np.where(kp <= qp, 1.0, 0.0).astype(np.float32)  # [128,128]
    ones = np.ones((128, 128), np.float32)
    zeros = np.zeros((128, 128), np.float32)
    mask0 = np.concatenate([tri, ones], axis=1)   # kb 2J vs [2J, 2J+1]
    mask1 = np.concatenate([zeros, tri], axis=1)  # kb 2J+1 vs [2J, 2J+1]
    return np.ascontiguousarray(
        np.concatenate([mask0, mask1, mask0, mask1], axis=1).astype(BF)
    )  # [128, 1024]


def _tile_hsT(hsT):
    """[896, 2048] -> [512, 3584]: row ss*128+p = concat over t of
    hsT[t*128+p, ss*512:(ss+1)*512], matching the SBUF projection layout."""
    out = np.empty((4 * 128, NH7 * 512), BF)
    for ss in range(4):
        blk = hsT[:, ss * 512 : (ss + 1) * 512].reshape(NH7, 128, 512)
        out[ss * 128 : (ss + 1) * 128, :] = (
            blk.transpose(1, 0, 2).reshape(128, NH7 * 512).astype(BF)
        )
    return np.ascontiguousarray(out)


_CONST_CACHE = None


def make_in_maps(hidden_states, wq, bq, wk, bk, wv, bv, wo):
    global _CONST_CACHE
    if _CONST_CACHE is None:
        cosd, sind = _rope_tables()
        _CONST_CACHE = (cosd, sind, _masks())
    cosd, sind, maskD = _CONST_CACHE
    # the tiled hidden states are shared by the 4 cores of a batch
    hs_tiled = [_tile_hsT(hidden_states[b].T) for b in range(B)]
    in_maps = []
    for core in range(8):
        b, kv, half = core // 4, (core % 4) // 2, core % 2
        if half == 0:
            slots = [kv * 7 + 0, kv * 7 + 1, kv * 7 + 2, kv * 7 + 3]
            dup = []
        else:
            slots = [kv * 7 + 4, kv * 7 + 5, kv * 7 + 6, kv * 7 + 3]
            dup = [3]
        cols = np.concatenate([np.arange(h * 64, (h + 1) * 64) for h in slots])
        wq4 = np.ascontiguousarray(wq[:, cols].astype(BF))
        bq4 = np.ascontiguousarray(bq[cols].reshape(2, 128))
        wkv = np.ascontiguousarray(
            np.concatenate(
                [wk[:, kv * 64 : (kv + 1) * 64], wv[:, kv * 64 : (kv + 1) * 64]],
                axis=1,
            ).astype(BF)
        )
        bkv = np.ascontiguousarray(
            np.concatenate(
                [bk[kv * 64 : (kv + 1) * 64], bv[kv * 64 : (kv + 1) * 64]]
            ).reshape(1, 128)
        )
        wo4 = wo[cols, :].copy()
        for d in dup:
            wo4[d * 64 : (d + 1) * 64, :] = 0.0
        in_maps.append(
            {
                "hsT": hs_tiled[b],
                "wq4": wq4,
                "bq4": bq4,
                "wkv": wkv,
                "bkv": bkv,
                "wo4": np.ascontiguousarray(wo4.astype(BF)),
                "cosd": cosd,
                "sind": sind,
                "maskD": maskD,
            }
        )
    return in_maps


_NC_CACHE = None


def _get_program():
    global _NC_CACHE
    if _NC_CACHE is None:
        _NC_CACHE = build_program()
    return _NC_CACHE


def kernel(hidden_states, wq, bq, wk, bk, wv, bv, wo):
    hidden_states = np.asarray(hidden_states, np.float32)
    wq = np.asarray(wq, np.float32)
    bq = np.asarray(bq, np.float32)
    wk = np.asarray(wk, np.float32)
    bk = np.asarray(bk, np.float32)
    wv = np.asarray(wv, np.float32)
    bv = np.asarray(bv, np.float32)
    wo = np.asarray(wo, np.float32)

    nc = _get_program()
    in_maps = make_in_maps(hidden_states, wq, bq, wk, bk, wv, bv, wo)
    res = run_bass_kernel_spmd(nc, in_maps, list(range(8)))
    out = np.zeros((B, S, HIDDEN), np.float32)
    for core in range(8):
        out[core // 4] += res.results[core]["out"]
    return out
